# revision 24
# baseline (speedup 1.0000x reference)
"""Bidirectional char-LSTM final-hidden kernel for Trainium2 (8 NeuronCores).

Strategy
--------
Data-parallel over words with length-equalized sharding:
  * Words are bucketed by length; each bucket is padded with dummy words to a
    multiple of 16 so every core receives an identical per-length histogram.
    This makes the (statically unrolled) Bass program identical across the 8
    cores while each core processes its own 1/8 of the words.
  * Per core, words are sorted by length ascending and packed into "groups"
    of <=512 columns.  The LSTM recurrence for a group runs only
    max-length-in-group steps, and every instruction's access pattern is
    narrowed to the live (not-yet-finished / already-started) column suffix,
    so ~zero wasted work on the ragged lengths.

Per step, gates are computed transposed ([H, words]) entirely on-chip:
    gates_p = E_p^T @ onehot_x_t  +  Whh_p^T @ h      (PE, PSUM f32 accum)
where E_p = W_ih^T + bias (one-hot rows sum to 1, so the bias rides along the
x matmul for free).  The host pre-transposes x into [C, words] bf16 blocks
(exact for one-hot data), so no on-device transposes are needed.
ACT does sigmoid/tanh straight out of PSUM; DVE does the cell updates in
bf16.  Forward outputs are captured per-step from the ending column range;
backward outputs are the full state after the last step.
"""

import sys

import numpy as np
import ml_dtypes

try:
    import concourse.bass as _probe  # noqa: F401
except ImportError:  # grading env without the repo on sys.path
    for _p in ("/root/.axon_site", "/root/.axon_site/_ro/trn_rl_repo",
               "/root/.axon_site/_ro/pypackages", "/opt/trn_rl_repo"):
        if _p not in sys.path:
            sys.path.append(_p)

import concourse.bass as bass
import concourse.bacc as bacc
import concourse.tile as tile
from concourse import mybir
from concourse.bass_utils import run_bass_kernel_spmd

N_WORDS = 16384
MAX_LEN = 16
N_CHARS = 128
HID = 128
NCORES = 8
GW_MAX = 512  # max group width (4 psum banks of gates per group-step)

_BF16 = ml_dtypes.bfloat16

# torch gate row order is (i, f, g, o); we lay gates out as (f, i, o, g) so
# the first step (which never uses f) can run one contiguous 3-gate sigmoid.
_PROG_GATE_TO_TORCH = (1, 0, 3, 2)


# ---------------------------------------------------------------------------
# host-side schedule construction
# ---------------------------------------------------------------------------

class Schedule:
    def __init__(self, lengths: np.ndarray):
        lengths = np.asarray(lengths).astype(np.int64)
        n = lengths.shape[0]
        assert n % NCORES == 0

        # bucket words by length; pad each bucket to a multiple of 2*NCORES
        # (divisible by NCORES for identical per-core histograms, and even
        # per core so every live-column offset is even -> bf16 slices stay
        # 4-byte aligned and DVE keeps its 2x perf mode)
        order = np.argsort(lengths, kind="stable")
        bucket = {}
        for L in range(1, MAX_LEN + 1):
            bucket[L] = order[lengths[order] == L]
        per_core_words = []  # per core: list of original indices (-1 dummy)
        for c in range(NCORES):
            per_core_words.append([])
        self.core_lens = None
        for L in range(1, MAX_LEN + 1):
            idx = bucket[L]
            cnt = len(idx)
            pad = (-cnt) % (2 * NCORES)
            tot = cnt + pad
            full = np.full(tot, -1, dtype=np.int64)
            full[:cnt] = idx
            per = tot // NCORES
            for c in range(NCORES):
                per_core_words[c].extend(full[c * per:(c + 1) * per].tolist())
        self.wpc = len(per_core_words[0])  # words per core (sorted ascending)
        assert all(len(w) == self.wpc for w in per_core_words)
        self.perm = np.array(per_core_words)  # [NCORES, wpc] original indices
        # per-core sorted length sequence -- identical for all cores
        lens_sorted = np.empty(self.wpc, dtype=np.int64)
        pos = 0
        for L in range(1, MAX_LEN + 1):
            cnt = len(bucket[L])
            per = (cnt + ((-cnt) % (2 * NCORES))) // NCORES
            lens_sorted[pos:pos + per] = L
            pos += per
        assert pos == self.wpc
        self.lens_sorted = lens_sorted

        # groups: remainder group first (shortest words), then 512-wide groups
        widths = []
        rem = self.wpc % GW_MAX
        if rem:
            widths.append(rem)
        widths.extend([GW_MAX] * (self.wpc // GW_MAX))
        self.gw = widths
        self.gbase = np.concatenate([[0], np.cumsum(widths)])[:-1].tolist()

        # per group: steps and live-prefix table a[t] = #words with len <= t
        self.S = []
        self.a = []
        for g, w in enumerate(widths):
            lg = lens_sorted[self.gbase[g]: self.gbase[g] + w]
            S = int(lg[-1])
            a = [int(np.searchsorted(lg, t, side="right")) for t in range(S + 1)]
            self.S.append(S)
            self.a.append(a)

        # x block layout: for (g, t): columns [a[g][t], gw[g]) at char t.
        # Within a group, blocks are packed interleaved from both ends of the
        # time axis ([t=0, t=S-1, t=1, t=S-2, ...]) so one small leading DMA
        # piece covers the first step of BOTH the fwd and bwd chains.
        self.xoff = []
        self.xsplit = []  # end of the leading piece per group
        off = 0
        for g, w in enumerate(widths):
            S = self.S[g]
            t_order = []
            i, j = 0, S - 1
            while i <= j:
                t_order.append(i)
                if j != i:
                    t_order.append(j)
                i += 1
                j -= 1
            offs = [0] * S
            for k, t in enumerate(t_order):
                offs[t] = off
                off += w - self.a[g][t]
                if k == min(1, S - 1):
                    self.xsplit.append(off)
            self.xoff.append(offs)
        self.totx = off

    def key(self):
        return (self.wpc, tuple(self.gw), tuple(self.S),
                tuple(tuple(a) for a in self.a))


# ---------------------------------------------------------------------------
# device program
# ---------------------------------------------------------------------------

def build_program(sch: Schedule):
    bf16 = mybir.dt.bfloat16
    f32 = mybir.dt.float32
    Sig = mybir.ActivationFunctionType.Sigmoid
    Tanh = mybir.ActivationFunctionType.Tanh
    mult = mybir.AluOpType.mult
    add = mybir.AluOpType.add

    nc = bacc.Bacc("TRN2", target_bir_lowering=False, debug=False,
                   num_devices=NCORES)
    xin = nc.dram_tensor("xin", [128, sch.totx], bf16, kind="ExternalInput")
    wts = nc.dram_tensor("wts", [128, 16 * 128], bf16, kind="ExternalInput")
    out = nc.dram_tensor("out", [2, 128, sch.wpc], f32, kind="ExternalOutput")

    ngroups = len(sch.gw)

    with tile.TileContext(nc) as tc:
        import contextlib
        with contextlib.ExitStack() as ctx:
            wpool = ctx.enter_context(tc.tile_pool(name="w", bufs=1))
            xpool = ctx.enter_context(tc.tile_pool(name="x", bufs=1))
            spool = ctx.enter_context(tc.tile_pool(name="state", bufs=1))
            opool = ctx.enter_context(tc.tile_pool(name="outs", bufs=1))
            gpool = ctx.enter_context(
                tc.tile_pool(name="gates", bufs=2, space="PSUM"))
            apool = ctx.enter_context(tc.tile_pool(name="acts", bufs=8))
            tpool = ctx.enter_context(tc.tile_pool(name="tmps", bufs=8))

            wtile = wpool.tile([128, 16 * 128], bf16, tag="wts")
            nc.sync.dma_start(wtile[:], wts[:])

            xtile = xpool.tile([128, sch.totx], bf16, tag="x")
            # all leading pieces (first fwd+bwd step blocks of every group)
            # are queued before any bulk piece: HWDGE drains its FIFO in
            # order, so every chain's first step unblocks within ~3us
            for g in range(ngroups):
                lo = sch.xoff[g][0]
                mid = sch.xsplit[g]
                nc.sync.dma_start(xtile[:, lo:mid], xin[:, lo:mid])
            for g in range(ngroups):
                hi = (sch.xoff[g + 1][0] if g + 1 < ngroups else sch.totx)
                mid = sch.xsplit[g]
                if mid < hi:
                    nc.sync.dma_start(xtile[:, mid:hi], xin[:, mid:hi])

            outf = opool.tile([128, sch.wpc], f32, tag="outf")
            outb = opool.tile([128, sch.wpc], f32, tag="outb")

            def wblk(d, typ, p):
                # weight block column range: d in {0 fwd,1 bwd},
                # typ 0 -> E (input proj + bias), 1 -> Whh^T
                i = (d * 8 + typ * 4 + p) * 128
                return wtile[:, i:i + 128]

            hs, cs = {}, {}
            for d in range(2):  # 0 = forward, 1 = backward
                for g, w in enumerate(sch.gw):
                    h_g = spool.tile([128, w], bf16, tag=f"h{d}_{g}")
                    c_g = spool.tile([128, w], bf16, tag=f"c{d}_{g}")
                    nc.vector.memset(h_g[:], 0.0)
                    nc.vector.memset(c_g[:], 0.0)
                    hs[d, g] = h_g
                    cs[d, g] = c_g

            maxS = max(sch.S)
            for s in range(maxS):
                for d in range(2):
                    for g, w in enumerate(sch.gw):
                        S = sch.S[g]
                        if s >= S:
                            continue
                        t = s if d == 0 else S - 1 - s
                        a = sch.a[g][t]
                        lo = a
                        n = w - a
                        h_g, c_g = hs[d, g], cs[d, g]

                        gp = gpool.tile([128, 4 * w], f32, tag="g")
                        gp3 = gp[:].rearrange("p (q w) -> p q w", q=4)
                        for p in range(4):
                            if s == 0 and p == 0:
                                continue  # f-gate unused at first step
                            nc.tensor.matmul(
                                gp[:, p * w + lo: (p + 1) * w],
                                wblk(d, 0, p),
                                xtile[:, sch.xoff[g][t]: sch.xoff[g][t] + n],
                                start=True, stop=(s == 0),
                            )
                            if s > 0:
                                nc.tensor.matmul(
                                    gp[:, p * w + lo: (p + 1) * w],
                                    wblk(d, 1, p),
                                    h_g[:, lo:w],
                                    start=False, stop=True,
                                )

                        sig = apool.tile([128, 4 * w], bf16, tag="sig")
                        sig3 = sig[:].rearrange("p (q w) -> p q w", q=4)
                        tg = apool.tile([128, w], bf16, tag="tg")
                        if s == 0:
                            nc.scalar.activation(
                                sig3[:, 1:4, lo:], gp3[:, 1:4, lo:], Sig)
                        else:
                            nc.scalar.activation(
                                sig3[:, :, lo:], gp3[:, :, lo:], Sig)
                        # tg = tanh(g) = 2*sigmoid(2g) - 1 (DVE, cheap)
                        nc.vector.tensor_scalar(
                            tg[:, lo:], sig3[:, 3, lo:], 2.0, -1.0, mult, add)

                        if s == 0:
                            nc.vector.tensor_tensor(
                                c_g[:, lo:], sig3[:, 1, lo:], tg[:, lo:], mult)
                        else:
                            t1 = tpool.tile([128, w], bf16, tag="t1")
                            t2 = tpool.tile([128, w], bf16, tag="t2")
                            nc.vector.tensor_tensor(
                                t1[:, lo:], sig3[:, 0, lo:], c_g[:, lo:], mult)
                            nc.vector.tensor_tensor(
                                t2[:, lo:], sig3[:, 1, lo:], tg[:, lo:], mult)
                            nc.vector.tensor_tensor(
                                c_g[:, lo:], t1[:, lo:], t2[:, lo:], add)

                        tc_ = apool.tile([128, w], bf16, tag="tc")
                        nc.scalar.activation(tc_[:, lo:], c_g[:, lo:], Tanh)
                        nc.vector.tensor_tensor(
                            h_g[:, lo:], sig3[:, 2, lo:], tc_[:, lo:], mult)

                        if d == 0:
                            # words of length s+1 end here
                            e0, e1 = sch.a[g][s], sch.a[g][s + 1]
                            if e1 > e0:
                                gb = sch.gbase[g]
                                nc.vector.tensor_copy(
                                    outf[:, gb + e0: gb + e1],
                                    h_g[:, e0:e1])

            # per-group output DMAs fire as soon as each group's pass ends
            for g, w in enumerate(sch.gw):
                gb = sch.gbase[g]
                nc.vector.tensor_copy(outb[:, gb: gb + w], hs[1, g][:, 0:w])
                nc.sync.dma_start(out[1][:, gb:gb + w], outb[:, gb:gb + w])
                nc.sync.dma_start(out[0][:, gb:gb + w], outf[:, gb:gb + w])

    nc.compile()
    return nc


# ---------------------------------------------------------------------------
# host data packing
# ---------------------------------------------------------------------------

def pack_weights(W_ih, W_hh, b_ih, b_hh):
    """[128, 8*128] bf16 for one direction: E blocks then Whh^T blocks."""
    b = (b_ih + b_hh).astype(np.float32)
    cols = []
    for p in range(4):
        m = _PROG_GATE_TO_TORCH[p]
        # g-gate (p==3) pre-scaled by 2: tanh(z) = 2*sigmoid(2z) - 1, so all
        # four gates go through one Sigmoid op on ACT.
        sc = 2.0 if p == 3 else 1.0
        E = W_ih[m * HID:(m + 1) * HID, :].T + b[m * HID:(m + 1) * HID][None, :]
        cols.append(sc * E)
    for p in range(4):
        m = _PROG_GATE_TO_TORCH[p]
        sc = 2.0 if p == 3 else 1.0
        cols.append(sc * W_hh[m * HID:(m + 1) * HID, :].T)
    return np.concatenate(cols, axis=1).astype(_BF16)


def pack_x(sch: Schedule, x: np.ndarray, core: int):
    """[128, totx] bf16: per-(group, t) live-column one-hot transposed."""
    perm = sch.perm[core]
    xs = np.zeros((sch.wpc, MAX_LEN, N_CHARS), dtype=np.float32)
    valid = perm >= 0
    xs[valid] = x[perm[valid]]
    outbuf = np.empty((128, sch.totx), dtype=_BF16)
    for g, w in enumerate(sch.gw):
        gb = sch.gbase[g]
        for t in range(sch.S[g]):
            a = sch.a[g][t]
            off = sch.xoff[g][t]
            blk = xs[gb + a: gb + w, t, :].T  # [128 chars, w-a words]
            outbuf[:, off: off + (w - a)] = blk.astype(_BF16)
    return outbuf


_CACHE = {}

# test-harness hooks: set TRACE=True before calling kernel() to profile; the
# BassKernelResults of the last run is stashed in LAST_RESULTS.
TRACE = False
LAST_RESULTS = None


def kernel(x, lengths, W_ih_f, W_hh_f, b_ih_f, b_hh_f,
           W_ih_b, W_hh_b, b_ih_b, b_hh_b):
    x = np.asarray(x, dtype=np.float32)
    lengths = np.asarray(lengths)

    sch = Schedule(lengths)
    key = sch.key()
    if key not in _CACHE:
        _CACHE[key] = build_program(sch)
    nc = _CACHE[key]

    wts = np.concatenate([
        pack_weights(np.asarray(W_ih_f, np.float32), np.asarray(W_hh_f, np.float32),
                     np.asarray(b_ih_f, np.float32), np.asarray(b_hh_f, np.float32)),
        pack_weights(np.asarray(W_ih_b, np.float32), np.asarray(W_hh_b, np.float32),
                     np.asarray(b_ih_b, np.float32), np.asarray(b_hh_b, np.float32)),
    ], axis=1)

    in_maps = []
    for c in range(NCORES):
        in_maps.append({"xin": pack_x(sch, x, c), "wts": wts})

    res = run_bass_kernel_spmd(nc, in_maps, core_ids=list(range(NCORES)),
                               trace=TRACE)
    global LAST_RESULTS
    LAST_RESULTS = res

    final = np.zeros((N_WORDS, 2 * HID), dtype=np.float32)
    for c in range(NCORES):
        o = res.results[c]["out"]  # [2, 128, wpc]
        full = np.concatenate([o[0].T, o[1].T], axis=1)  # [wpc, 256]
        perm = sch.perm[c]
        valid = perm >= 0
        final[perm[valid]] = full[valid]
    return final


# revision 26
# speedup vs baseline: 1.0089x; 1.0089x over previous
"""Bidirectional char-LSTM final-hidden kernel for Trainium2 (8 NeuronCores).

Strategy
--------
Data-parallel over words with length-equalized sharding:
  * Words are bucketed by length; each bucket is padded with dummy words to a
    multiple of 16 so every core receives an identical per-length histogram.
    This makes the (statically unrolled) Bass program identical across the 8
    cores while each core processes its own 1/8 of the words.
  * Per core, words are sorted by length ascending and packed into "groups"
    of <=512 columns.  The LSTM recurrence for a group runs only
    max-length-in-group steps, and every instruction's access pattern is
    narrowed to the live (not-yet-finished / already-started) column suffix,
    so ~zero wasted work on the ragged lengths.

Per step, gates are computed transposed ([H, words]) entirely on-chip:
    gates_p = E_p^T @ onehot_x_t  +  Whh_p^T @ h      (PE, PSUM f32 accum)
where E_p = W_ih^T + bias (one-hot rows sum to 1, so the bias rides along the
x matmul for free).  The host pre-transposes x into [C, words] bf16 blocks
(exact for one-hot data), so no on-device transposes are needed.
ACT does sigmoid/tanh straight out of PSUM; DVE does the cell updates in
bf16.  Forward outputs are captured per-step from the ending column range;
backward outputs are the full state after the last step.
"""

import sys

import numpy as np
import ml_dtypes

try:
    import concourse.bass as _probe  # noqa: F401
except ImportError:  # grading env without the repo on sys.path
    for _p in ("/root/.axon_site", "/root/.axon_site/_ro/trn_rl_repo",
               "/root/.axon_site/_ro/pypackages", "/opt/trn_rl_repo"):
        if _p not in sys.path:
            sys.path.append(_p)

import concourse.bass as bass
import concourse.bacc as bacc
import concourse.tile as tile
from concourse import mybir
from concourse.bass_utils import run_bass_kernel_spmd

N_WORDS = 16384
MAX_LEN = 16
N_CHARS = 128
HID = 128
NCORES = 8
GW_MAX = 256  # max group width (2 psum banks of gates per group-step)

_BF16 = ml_dtypes.bfloat16

# torch gate row order is (i, f, g, o); we lay gates out as (f, i, o, g) so
# the first step (which never uses f) can run one contiguous 3-gate sigmoid.
_PROG_GATE_TO_TORCH = (1, 0, 3, 2)


# ---------------------------------------------------------------------------
# host-side schedule construction
# ---------------------------------------------------------------------------

class Schedule:
    def __init__(self, lengths: np.ndarray):
        lengths = np.asarray(lengths).astype(np.int64)
        n = lengths.shape[0]
        assert n % NCORES == 0

        # bucket words by length; pad each bucket to a multiple of 2*NCORES
        # (divisible by NCORES for identical per-core histograms, and even
        # per core so every live-column offset is even -> bf16 slices stay
        # 4-byte aligned and DVE keeps its 2x perf mode)
        order = np.argsort(lengths, kind="stable")
        bucket = {}
        for L in range(1, MAX_LEN + 1):
            bucket[L] = order[lengths[order] == L]
        per_core_words = []  # per core: list of original indices (-1 dummy)
        for c in range(NCORES):
            per_core_words.append([])
        self.core_lens = None
        for L in range(1, MAX_LEN + 1):
            idx = bucket[L]
            cnt = len(idx)
            pad = (-cnt) % (2 * NCORES)
            tot = cnt + pad
            full = np.full(tot, -1, dtype=np.int64)
            full[:cnt] = idx
            per = tot // NCORES
            for c in range(NCORES):
                per_core_words[c].extend(full[c * per:(c + 1) * per].tolist())
        self.wpc = len(per_core_words[0])  # words per core (sorted ascending)
        assert all(len(w) == self.wpc for w in per_core_words)
        self.perm = np.array(per_core_words)  # [NCORES, wpc] original indices
        # per-core sorted length sequence -- identical for all cores
        lens_sorted = np.empty(self.wpc, dtype=np.int64)
        pos = 0
        for L in range(1, MAX_LEN + 1):
            cnt = len(bucket[L])
            per = (cnt + ((-cnt) % (2 * NCORES))) // NCORES
            lens_sorted[pos:pos + per] = L
            pos += per
        assert pos == self.wpc
        self.lens_sorted = lens_sorted

        # groups: remainder group first (shortest words), then 512-wide groups
        widths = []
        rem = self.wpc % GW_MAX
        if rem:
            widths.append(rem)
        widths.extend([GW_MAX] * (self.wpc // GW_MAX))
        self.gw = widths
        self.gbase = np.concatenate([[0], np.cumsum(widths)])[:-1].tolist()

        # per group: steps and live-prefix table a[t] = #words with len <= t
        self.S = []
        self.a = []
        for g, w in enumerate(widths):
            lg = lens_sorted[self.gbase[g]: self.gbase[g] + w]
            S = int(lg[-1])
            a = [int(np.searchsorted(lg, t, side="right")) for t in range(S + 1)]
            self.S.append(S)
            self.a.append(a)

        # x block layout: for (g, t): columns [a[g][t], gw[g]) at char t.
        # Within a group, blocks are packed interleaved from both ends of the
        # time axis ([t=0, t=S-1, t=1, t=S-2, ...]) so one small leading DMA
        # piece covers the first step of BOTH the fwd and bwd chains.
        self.xoff = []
        self.xsplit = []  # end of the leading piece per group
        off = 0
        for g, w in enumerate(widths):
            S = self.S[g]
            t_order = []
            i, j = 0, S - 1
            while i <= j:
                t_order.append(i)
                if j != i:
                    t_order.append(j)
                i += 1
                j -= 1
            offs = [0] * S
            for k, t in enumerate(t_order):
                offs[t] = off
                off += w - self.a[g][t]
                if k == min(1, S - 1):
                    self.xsplit.append(off)
            self.xoff.append(offs)
        self.totx = off

    def key(self):
        return (self.wpc, tuple(self.gw), tuple(self.S),
                tuple(tuple(a) for a in self.a))


# ---------------------------------------------------------------------------
# device program
# ---------------------------------------------------------------------------

def build_program(sch: Schedule):
    bf16 = mybir.dt.bfloat16
    f32 = mybir.dt.float32
    Sig = mybir.ActivationFunctionType.Sigmoid
    Tanh = mybir.ActivationFunctionType.Tanh
    mult = mybir.AluOpType.mult
    add = mybir.AluOpType.add

    nc = bacc.Bacc("TRN2", target_bir_lowering=False, debug=False,
                   num_devices=NCORES)
    xin = nc.dram_tensor("xin", [128, sch.totx], bf16, kind="ExternalInput")
    wts = nc.dram_tensor("wts", [128, 16 * 128], bf16, kind="ExternalInput")
    out = nc.dram_tensor("out", [2, 128, sch.wpc], f32, kind="ExternalOutput")

    ngroups = len(sch.gw)

    with tile.TileContext(nc) as tc:
        import contextlib
        with contextlib.ExitStack() as ctx:
            wpool = ctx.enter_context(tc.tile_pool(name="w", bufs=1))
            xpool = ctx.enter_context(tc.tile_pool(name="x", bufs=1))
            spool = ctx.enter_context(tc.tile_pool(name="state", bufs=1))
            opool = ctx.enter_context(tc.tile_pool(name="outs", bufs=1))
            gpool = ctx.enter_context(
                tc.tile_pool(name="gates", bufs=4, space="PSUM"))
            apool = ctx.enter_context(tc.tile_pool(name="acts", bufs=8))
            tpool = ctx.enter_context(tc.tile_pool(name="tmps", bufs=8))

            wtile = wpool.tile([128, 16 * 128], bf16, tag="wts")
            nc.sync.dma_start(wtile[:], wts[:])

            xtile = xpool.tile([128, sch.totx], bf16, tag="x")
            # all leading pieces (first fwd+bwd step blocks of every group)
            # are queued before any bulk piece: HWDGE drains its FIFO in
            # order, so every chain's first step unblocks within ~3us
            for g in range(ngroups):
                lo = sch.xoff[g][0]
                mid = sch.xsplit[g]
                nc.sync.dma_start(xtile[:, lo:mid], xin[:, lo:mid])
            for g in range(ngroups):
                hi = (sch.xoff[g + 1][0] if g + 1 < ngroups else sch.totx)
                mid = sch.xsplit[g]
                if mid < hi:
                    nc.sync.dma_start(xtile[:, mid:hi], xin[:, mid:hi])

            outf = opool.tile([128, sch.wpc], f32, tag="outf")
            outb = opool.tile([128, sch.wpc], f32, tag="outb")

            def wblk(d, typ, p):
                # weight block column range: d in {0 fwd,1 bwd},
                # typ 0 -> E (input proj + bias), 1 -> Whh^T
                i = (d * 8 + typ * 4 + p) * 128
                return wtile[:, i:i + 128]

            hs, cs = {}, {}
            for d in range(2):  # 0 = forward, 1 = backward
                for g, w in enumerate(sch.gw):
                    h_g = spool.tile([128, w], bf16, tag=f"h{d}_{g}")
                    c_g = spool.tile([128, w], bf16, tag=f"c{d}_{g}")
                    nc.vector.memset(h_g[:], 0.0)
                    nc.vector.memset(c_g[:], 0.0)
                    hs[d, g] = h_g
                    cs[d, g] = c_g

            maxS = max(sch.S)
            for s in range(maxS):
                for d in range(2):
                    for g, w in enumerate(sch.gw):
                        S = sch.S[g]
                        if s >= S:
                            continue
                        t = s if d == 0 else S - 1 - s
                        a = sch.a[g][t]
                        lo = a
                        n = w - a
                        h_g, c_g = hs[d, g], cs[d, g]

                        gp = gpool.tile([128, 4 * w], f32, tag="g")
                        gp3 = gp[:].rearrange("p (q w) -> p q w", q=4)
                        for p in range(4):
                            if s == 0 and p == 0:
                                continue  # f-gate unused at first step
                            nc.tensor.matmul(
                                gp[:, p * w + lo: (p + 1) * w],
                                wblk(d, 0, p),
                                xtile[:, sch.xoff[g][t]: sch.xoff[g][t] + n],
                                start=True, stop=(s == 0),
                            )
                            if s > 0:
                                nc.tensor.matmul(
                                    gp[:, p * w + lo: (p + 1) * w],
                                    wblk(d, 1, p),
                                    h_g[:, lo:w],
                                    start=False, stop=True,
                                )

                        sig = apool.tile([128, 4 * w], bf16, tag="sig")
                        sig3 = sig[:].rearrange("p (q w) -> p q w", q=4)
                        tg = apool.tile([128, w], bf16, tag="tg")
                        if s == 0:
                            nc.scalar.activation(
                                sig3[:, 1:4, lo:], gp3[:, 1:4, lo:], Sig)
                        else:
                            nc.scalar.activation(
                                sig3[:, :, lo:], gp3[:, :, lo:], Sig)
                        # tg = tanh(g) = 2*sigmoid(2g) - 1 (DVE, cheap)
                        nc.vector.tensor_scalar(
                            tg[:, lo:], sig3[:, 3, lo:], 2.0, -1.0, mult, add)

                        if s == 0:
                            nc.vector.tensor_tensor(
                                c_g[:, lo:], sig3[:, 1, lo:], tg[:, lo:], mult)
                        else:
                            t1 = tpool.tile([128, w], bf16, tag="t1")
                            t2 = tpool.tile([128, w], bf16, tag="t2")
                            nc.vector.tensor_tensor(
                                t1[:, lo:], sig3[:, 0, lo:], c_g[:, lo:], mult)
                            nc.vector.tensor_tensor(
                                t2[:, lo:], sig3[:, 1, lo:], tg[:, lo:], mult)
                            nc.vector.tensor_tensor(
                                c_g[:, lo:], t1[:, lo:], t2[:, lo:], add)

                        tc_ = apool.tile([128, w], bf16, tag="tc")
                        nc.scalar.activation(tc_[:, lo:], c_g[:, lo:], Tanh)
                        nc.vector.tensor_tensor(
                            h_g[:, lo:], sig3[:, 2, lo:], tc_[:, lo:], mult)

                        if d == 0:
                            # words of length s+1 end here
                            e0, e1 = sch.a[g][s], sch.a[g][s + 1]
                            if e1 > e0:
                                gb = sch.gbase[g]
                                nc.vector.tensor_copy(
                                    outf[:, gb + e0: gb + e1],
                                    h_g[:, e0:e1])

            # per-group output DMAs fire as soon as each group's pass ends
            for g, w in enumerate(sch.gw):
                gb = sch.gbase[g]
                nc.vector.tensor_copy(outb[:, gb: gb + w], hs[1, g][:, 0:w])
                nc.sync.dma_start(out[1][:, gb:gb + w], outb[:, gb:gb + w])
                nc.sync.dma_start(out[0][:, gb:gb + w], outf[:, gb:gb + w])

    nc.compile()
    return nc


# ---------------------------------------------------------------------------
# host data packing
# ---------------------------------------------------------------------------

def pack_weights(W_ih, W_hh, b_ih, b_hh):
    """[128, 8*128] bf16 for one direction: E blocks then Whh^T blocks."""
    b = (b_ih + b_hh).astype(np.float32)
    cols = []
    for p in range(4):
        m = _PROG_GATE_TO_TORCH[p]
        # g-gate (p==3) pre-scaled by 2: tanh(z) = 2*sigmoid(2z) - 1, so all
        # four gates go through one Sigmoid op on ACT.
        sc = 2.0 if p == 3 else 1.0
        E = W_ih[m * HID:(m + 1) * HID, :].T + b[m * HID:(m + 1) * HID][None, :]
        cols.append(sc * E)
    for p in range(4):
        m = _PROG_GATE_TO_TORCH[p]
        sc = 2.0 if p == 3 else 1.0
        cols.append(sc * W_hh[m * HID:(m + 1) * HID, :].T)
    return np.concatenate(cols, axis=1).astype(_BF16)


def pack_x(sch: Schedule, x: np.ndarray, core: int):
    """[128, totx] bf16: per-(group, t) live-column one-hot transposed."""
    perm = sch.perm[core]
    xs = np.zeros((sch.wpc, MAX_LEN, N_CHARS), dtype=np.float32)
    valid = perm >= 0
    xs[valid] = x[perm[valid]]
    outbuf = np.empty((128, sch.totx), dtype=_BF16)
    for g, w in enumerate(sch.gw):
        gb = sch.gbase[g]
        for t in range(sch.S[g]):
            a = sch.a[g][t]
            off = sch.xoff[g][t]
            blk = xs[gb + a: gb + w, t, :].T  # [128 chars, w-a words]
            outbuf[:, off: off + (w - a)] = blk.astype(_BF16)
    return outbuf


_CACHE = {}

# test-harness hooks: set TRACE=True before calling kernel() to profile; the
# BassKernelResults of the last run is stashed in LAST_RESULTS.
TRACE = False
LAST_RESULTS = None


def kernel(x, lengths, W_ih_f, W_hh_f, b_ih_f, b_hh_f,
           W_ih_b, W_hh_b, b_ih_b, b_hh_b):
    x = np.asarray(x, dtype=np.float32)
    lengths = np.asarray(lengths)

    sch = Schedule(lengths)
    key = sch.key()
    if key not in _CACHE:
        _CACHE[key] = build_program(sch)
    nc = _CACHE[key]

    wts = np.concatenate([
        pack_weights(np.asarray(W_ih_f, np.float32), np.asarray(W_hh_f, np.float32),
                     np.asarray(b_ih_f, np.float32), np.asarray(b_hh_f, np.float32)),
        pack_weights(np.asarray(W_ih_b, np.float32), np.asarray(W_hh_b, np.float32),
                     np.asarray(b_ih_b, np.float32), np.asarray(b_hh_b, np.float32)),
    ], axis=1)

    in_maps = []
    for c in range(NCORES):
        in_maps.append({"xin": pack_x(sch, x, c), "wts": wts})

    res = run_bass_kernel_spmd(nc, in_maps, core_ids=list(range(NCORES)),
                               trace=TRACE)
    global LAST_RESULTS
    LAST_RESULTS = res

    final = np.zeros((N_WORDS, 2 * HID), dtype=np.float32)
    for c in range(NCORES):
        o = res.results[c]["out"]  # [2, 128, wpc]
        full = np.concatenate([o[0].T, o[1].T], axis=1)  # [wpc, 256]
        perm = sch.perm[c]
        valid = perm >= 0
        final[perm[valid]] = full[valid]
    return final
